# revision 19
# baseline (speedup 1.0000x reference)
"""CTC loss (keras ctc_batch_cost semantics) on Trainium2, 8-core data parallel.

Bidirectional packed wavefront (per core, 64 examples on 128 partitions):
  Linear-domain CTC with per-step rescale K (p' = K*p, loss = T*log K -
  log P). The lattice is split at t = T/2: partitions 0-63 run the FORWARD
  wavefront over t in [0, 256) (columns bl_0, l_0, bl_1, ...), partitions
  64-127 run the BACKWARD wavefront over reversed time tau = 511-t
  (columns bl_48, l_47, bl_47, ...). Both recurrences have the identical
  "atilde" scan form (state = pbsh*state + lprev; x = atilde + (m-1)*lprev;
  l-scan state = (x + state)*pg), so each wavefront step is ONE [128, ~233]
  DVE tensor_tensor_scan pair + one scalar_tensor_tensor — half the serial
  elements per scan of the unidirectional version. Column i is zero for
  k < i in both directions (head pruning).

  Merge at the cut: P = sum_{s'} I_{s'} * bhat_{s'}(256), where I_{s'} is
  the forward pre-multiply inflow at t=256 (atilde/x/l slots 256) and
  bhat_{s'}(256) comes from the backward tiles (slot 255 * pb[256] for
  blanks, raw slot 256 for labels). Boundary values are packed by tiny
  scalar-engine copies during the wavefront, moved across partitions with
  one SBUF-to-SBUF DMA, and combined with two strided TT + reduce pairs.

  Data movement: the host gathers the per-column label probabilities and the
  blank row directly (pg[p, j, k] = bf16(K * y_pred[b, t, lab]) in wavefront
  layout, h=0: t=k forward / h=1: t=511-k backward), so the device streams
  plain contiguous DMAs — no indirect gathers. The pg mega-tile is loaded in
  8-column chunks so the first columns' data lands ahead of the wavefront and
  the rest streams underneath it.

Shapes are hardcoded for B=512, T=512, C=128, L=48 (S=97), 8 cores.
"""

import sys

if "/opt/trn_rl_repo" not in sys.path:
    sys.path.insert(0, "/opt/trn_rl_repo")

import math

import ml_dtypes
import numpy as np

import concourse.bacc as bacc
import concourse.bass as bass
import concourse.tile as tile
from concourse import mybir
from concourse.bass_utils import run_bass_kernel_spmd

NCORES = 8
B, T, C, L = 512, 512, 128, 48
BL = B // NCORES  # 64 examples per core
BLANK = C - 1
H = T // 2  # 256: the fwd/bwd cut
NS = H + 1  # 257 slots per column tile (slot k = value at time k-1)
K = 75.0  # per-step rescale; log K ~= 4.317
F32 = mybir.dt.float32
BF16 = mybir.dt.bfloat16
I32 = mybir.dt.int32
ALU = mybir.AluOpType
ACTF = mybir.ActivationFunctionType

PGCHUNK = 8  # label columns per pg DMA
M = 97  # chunk-boundary slot: DVE scans slots [j, M), Pool scans [M, 257)


def build_ctc_program(nc: bass.Bass):
    pgd = nc.dram_tensor("pg", [2 * BL, L * H], BF16, kind="ExternalInput").ap()
    pbd = nc.dram_tensor("pb", [2 * BL, H], BF16, kind="ExternalInput").ap()
    mskd = nc.dram_tensor("msk", [2 * BL, L], F32, kind="ExternalInput").ap()
    out = nc.dram_tensor("out", [BL, 1], F32, kind="ExternalOutput").ap()

    with tile.TileContext(nc) as tc:
        _ctc_body(nc, tc, pgd, pbd, mskd, out)
    return out


def _ctc_body(nc, tc, pgd, pbd, mskd, out):
    P2 = 2 * BL  # 128 partitions: fwd examples | bwd examples

    with (
        tc.tile_pool(name="const", bufs=1) as cpool,
        tc.tile_pool(name="fin", bufs=1) as fpool,
    ):
        # ---- inputs ------------------------------------------------------
        # pbshc[p, k] = blank prob at step k-1 of this half; slot 0 = 1.0
        # (first in queue: scanA_0 only needs this)
        pbshc = cpool.tile([P2, NS], BF16)
        nc.sync.dma_start(out=pbshc[:, 1:NS], in_=pbd[:, :])
        nc.gpsimd.memset(pbshc[:, 0:1], 1.0)

        # pg mega tile: column j at [:, j*H:(j+1)*H]; column 0 goes out on
        # the Activation engine's DMA queue, concurrent with pb on SP's, so
        # scanL_0 starts ASAP; the rest (and msk, first needed by the
        # column-1 Act multiply) stream under the wavefront on SP's queue
        pgm = cpool.tile([P2, L * H], BF16)
        nc.scalar.dma_start(out=pgm[:, 0:H], in_=pgd[:, 0:H])

        # mc[p, j] = m - 1 in {0,-1}: x = atilde + (m-1)*lprev (skip correction)
        mc = cpool.tile([P2, L], F32)
        nc.sync.dma_start(out=mc[:], in_=mskd[:, :])

        c0 = 1
        for w in (1, 1, 1, 4, 8, 8, 8, 8, 8):
            nc.sync.dma_start(
                out=pgm[:, c0 * H : (c0 + w) * H],
                in_=pgd[:, c0 * H : (c0 + w) * H],
            )
            c0 += w
        assert c0 == L

        # touch Ln once so its table loads during startup slack
        warm = cpool.tile([BL, 1], F32)
        nc.vector.memset(warm[:], 1.0)
        nc.scalar.activation(out=warm[:], in_=warm[:], func=ACTF.Ln)

        # ---- column storage: mega tiles (slot 256 read back at the merge)
        amega = cpool.tile([P2, (L + 1) * NS], BF16)
        lmega = cpool.tile([P2, L * NS], BF16)
        xmega = cpool.tile([P2, L * NS], BF16)
        zcol = cpool.tile([P2, NS], BF16)
        nc.gpsimd.memset(zcol[:], 0.0)
        # bwpack[64:128, i] = backward boundary values, packed in column
        # order: [pb256*btilde_bl 49 | bhat_l 48] (pb256 folded into the
        # blank copies via per-partition Act scale; scale APs must be f32)
        bwpack = cpool.tile([P2, 2 * L + 1], BF16)
        pb256f = cpool.tile([P2, 1], F32)
        nc.vector.tensor_copy(out=pb256f[:], in_=pbshc[:, NS - 1 : NS])

        # ---- packed bidirectional wavefront ------------------------------
        # Per column: DVE does scanA, a 2x-mode TT add, scanL. The skip
        # correction's multiply (mcl = mc*lprev) runs on the idle Activation
        # engine (per-partition scale), overlapped under DVE's scanA, so the
        # DVE chain per column is scan + cheap TT + scan instead of
        # scan + 1x STT + scan. Column 0 has lprev = 0, so its x IS atilde
        # and both the TT and the Act multiply are skipped.
        lprev = zcol
        for j in range(L + 1):
            # top: fwd atilde_j[k] = a_j(k-1) + l_{j-1}(k-1)
            # bottom: bwd btilde for bl_{48-j} (same recurrence, reversed data)
            acol = amega[:, j * NS : (j + 1) * NS]
            nc.vector.tensor_tensor_scan(
                out=acol[:, j:NS], data0=pbshc[:, j:NS], data1=lprev[:, j:NS],
                initial=1.0 if j == 0 else 0.0, op0=ALU.mult, op1=ALU.add,
            )
            # bwd blank boundary: pb256 * btilde_bl_{48-j}[255]
            nc.scalar.activation(
                out=bwpack[BL:P2, L - j : L - j + 1],
                in_=acol[BL:P2, NS - 2 : NS - 1], func=ACTF.Copy,
                scale=pb256f[BL:P2, :],
            )
            if j == L:
                break

            # x = atilde + (m-1)*lprev (skip correction)
            if j == 0:
                x = acol
                # the merge reads x_0[256] from xmega; mirror it there
                nc.scalar.activation(
                    out=xmega[0:BL, NS - 1 : NS],
                    in_=acol[0:BL, NS - 1 : NS], func=ACTF.Copy,
                )
            else:
                x = xmega[:, j * NS : (j + 1) * NS]
                nc.vector.tensor_tensor(
                    out=x[:, j:NS], in0=x[:, j:NS], in1=acol[:, j:NS],
                    op=ALU.add,
                )

            # top: fwd l_j; bottom: bwd l_{47-j}
            lcol = lmega[:, j * NS : (j + 1) * NS]
            nc.vector.tensor_tensor_scan(
                out=lcol[:, j + 1 : NS], data0=x[:, j:H],
                data1=pgm[:, j * H + j : (j + 1) * H],
                initial=0.0, op0=ALU.add, op1=ALU.mult,
            )
            # bwd label boundary: bhat_l_{47-j}(tau=255), raw slot 256
            nc.scalar.activation(
                out=bwpack[BL:P2, L + 1 + L - 1 - j : L + 1 + L - j],
                in_=lcol[BL:P2, NS - 1 : NS], func=ACTF.Copy,
            )
            # mcl for the NEXT column: (m-1)*l_j on the Activation engine,
            # hidden under the next scanA
            if j + 1 <= L - 1:
                xn = xmega[:, (j + 1) * NS : (j + 2) * NS]
                nc.scalar.activation(
                    out=xn[:, j + 1 : NS], in_=lcol[:, j + 1 : NS],
                    func=ACTF.Copy, scale=mc[:, j + 1 : j + 2],
                )
            if j == L - 1:
                # hoist the merge's label-inflow combine into the gap before
                # the final blank scan: ul_j = l_j(255) + x_j[256]
                l3w = lmega[:].rearrange("p (j s) -> p j s", s=NS)
                x3w = xmega[:].rearrange("p (j s) -> p j s", s=NS)
                ul = fpool.tile([BL, L], F32)
                ul3 = ul[:].rearrange("p (j s) -> p j s", s=1)
                nc.vector.tensor_tensor(
                    out=ul3[:], in0=l3w[0:BL, :, NS - 1 : NS],
                    in1=x3w[0:BL, :, NS - 1 : NS], op=ALU.add,
                )
            lprev = lcol

        # ---- merge at the cut --------------------------------------------
        # move backward boundary vector to the forward partitions: DVE
        # stream shuffle (identity mask, base-partition-64 view) instead of
        # an SBUF-to-SBUF DMA round trip (saves ~1.7us of DMA+sem latency)
        NB = 2 * L + 1  # 97: [pb256*btilde_bl 49 | bhat_l 48]
        shuf = fpool.tile([BL, NB], BF16)
        nc.vector.stream_shuffle(
            out=shuf[:], in_=bwpack[BL:P2, :], mask=list(range(32))
        )

        a3 = amega[:].rearrange("p (j s) -> p j s", s=NS)
        s3 = shuf[:].rearrange("p (j s) -> p j s", s=1)
        ul3 = ul[:].rearrange("p (j s) -> p j s", s=1)

        # prod[:, 0:49] = atilde_j[256] * (pb256*btilde_bl_j[255])
        # prod[:, 49:97] = (l_j(255) + x_j[256]) * bhat_l_j(255)
        prod = fpool.tile([BL, NB], F32)
        p3 = prod[:].rearrange("p (j s) -> p j s", s=1)
        nc.vector.tensor_tensor(
            out=p3[:, 0 : L + 1, :], in0=a3[0:BL, :, NS - 1 : NS],
            in1=s3[:, 0 : L + 1, :], op=ALU.mult,
        )
        nc.vector.tensor_tensor(
            out=p3[:, L + 1 : NB, :], in0=ul3[:], in1=s3[:, L + 1 : NB, :],
            op=ALU.mult,
        )
        # P = sum of all 97 terms
        z = fpool.tile([BL, 1], F32)
        nc.vector.tensor_reduce(out=z[:], in_=prod[:], axis=mybir.AxisListType.X, op=ALU.add)

        # ---- finalize: loss = T*log K - log P ----------------------------
        logz = fpool.tile([BL, 1], F32)
        nc.scalar.activation(out=logz[:], in_=z[:], func=ACTF.Ln)
        loss = fpool.tile([BL, 1], F32)
        nc.scalar.activation(
            out=loss[:], in_=logz[:], func=ACTF.Copy,
            scale=-1.0, bias=float(T * math.log(K)),
        )
        nc.sync.dma_start(out=out[:, :], in_=loss[:])


_CACHE: dict = {}


def _get_program():
    if "nc" not in _CACHE:
        nc = bacc.Bacc("TRN2", target_bir_lowering=False, debug=False)
        build_ctc_program(nc)
        nc.compile()
        _CACHE["nc"] = nc
    return _CACHE["nc"]


def kernel(y_true: np.ndarray, y_pred: np.ndarray) -> np.ndarray:
    nc = _get_program()
    lab = np.ascontiguousarray(np.asarray(y_true).astype(np.int32))  # [B, L]
    yp = np.asarray(y_pred, dtype=np.float32)  # [B, T, C]
    # input conditioning: constant K rescale folded into the bf16 quantization,
    # fwd half t=0..255 / reversed bwd half t=511..256 per partition group
    yp2 = (K * yp).astype(ml_dtypes.bfloat16)
    fwd = yp2[:, :H, :]  # [B, 256, C], time-major
    bwd = yp2[:, H:, :][:, ::-1, :]

    labc = lab.reshape(NCORES, BL, L)
    # host-side gather: pg_top[core, b, j, k] = fwd[b, k, lab[b, j]]
    fwdc = fwd.reshape(NCORES, BL, H, C)
    bwdc = bwd.reshape(NCORES, BL, H, C)
    idx_top = labc[:, :, None, :]  # [NC, BL, 1, L]
    idx_bot = labc[:, :, ::-1][:, :, None, :]
    pg_top = np.take_along_axis(fwdc, idx_top, axis=3)  # [NC, BL, H, L]
    pg_bot = np.take_along_axis(bwdc, idx_bot, axis=3)
    pg_top = pg_top.transpose(0, 1, 3, 2)  # [NC, BL, L, H]
    pg_bot = pg_bot.transpose(0, 1, 3, 2)

    pb_top = fwdc[:, :, :, BLANK]  # [NC, BL, H]
    pb_bot = bwdc[:, :, :, BLANK]

    m = np.zeros((B, L), dtype=np.float32)
    m[:, 1:] = (lab[:, 1:] != lab[:, :-1]).astype(np.float32)
    mc_top = m - 1.0  # mneg_j (col 0 unused: lprev_0 = 0)
    mc_bot = np.zeros((B, L), dtype=np.float32)
    mc_bot[:, 1:] = m[:, :0:-1] - 1.0  # col j>=1: m[:, L-j] - 1
    mct = mc_top.reshape(NCORES, BL, L)
    mcb = mc_bot.reshape(NCORES, BL, L)

    in_maps = [
        {
            "pg": np.ascontiguousarray(
                np.concatenate([pg_top[c], pg_bot[c]], axis=0).reshape(2 * BL, L * H)
            ),
            "pb": np.ascontiguousarray(
                np.concatenate([pb_top[c], pb_bot[c]], axis=0)
            ),
            "msk": np.ascontiguousarray(np.concatenate([mct[c], mcb[c]], axis=0)),
        }
        for c in range(NCORES)
    ]
    res = run_bass_kernel_spmd(nc, in_maps, list(range(NCORES)))
    return np.concatenate([res.results[c]["out"] for c in range(NCORES)], axis=0)


# revision 22
# speedup vs baseline: 1.0008x; 1.0008x over previous
"""CTC loss (keras ctc_batch_cost semantics) on Trainium2, 8-core data parallel.

Bidirectional packed wavefront (per core, 64 examples on 128 partitions):
  Linear-domain CTC with per-step rescale K (p' = K*p, loss = T*log K -
  log P). The lattice is split at t = T/2: partitions 0-63 run the FORWARD
  wavefront over t in [0, 256) (columns bl_0, l_0, bl_1, ...), partitions
  64-127 run the BACKWARD wavefront over reversed time tau = 511-t
  (columns bl_48, l_47, bl_47, ...). Both recurrences have the identical
  "atilde" scan form (state = pbsh*state + lprev; x = atilde + (m-1)*lprev;
  l-scan state = (x + state)*pg), so each wavefront step is two [128, ~233]
  DVE tensor_tensor_scans plus the skip correction. Column i is zero for
  k < i in both directions (head pruning).

  Skip correction: the multiply mcl = (m-1)*l_j runs on the otherwise-idle
  Activation engine (Copy with per-partition scale), overlapped under the
  next column's blank scan; DVE then only pays a 2x-mode TensorTensor add
  (x = mcl + atilde) instead of a 1x scalar_tensor_tensor.

  Merge at the cut: P = sum_{s'} I_{s'} * bhat_{s'}(256), where I_{s'} is
  the forward pre-multiply inflow at t=256 (atilde/x/l slots 256) and
  bhat_{s'}(256) comes from the backward tiles (slot 255 * pb256 for
  blanks — pb256 folded into the boundary copies via Act scale — and raw
  slot 256 for labels). Boundary values are packed by tiny Act copies during
  the wavefront, moved across partitions with one DVE StreamShuffle
  (identity mask over a base-partition-64 view; no DMA round trip), and
  combined with two strided TTs and a single reduce.

  Data movement: the host gathers the per-column label probabilities and the
  blank row directly (pg[p, j, k] = bf16(K * y_pred[b, t, lab]) in wavefront
  layout, h=0: t=k forward / h=1: t=511-k backward), so the device streams
  plain contiguous DMAs — no indirect gathers (a Pool-engine SWDGE gather
  costs ~1us per column and was the original bottleneck). pb and pg column 0
  are queued first so the wavefront starts ~3us in; the remaining columns
  stream under it in growing chunks.

Shapes are hardcoded for B=512, T=512, C=128, L=48 (S=97), 8 cores.
"""

import sys

if "/opt/trn_rl_repo" not in sys.path:
    sys.path.insert(0, "/opt/trn_rl_repo")

import math

import ml_dtypes
import numpy as np

import concourse.bacc as bacc
import concourse.bass as bass
import concourse.tile as tile
from concourse import mybir
from concourse.bass_utils import run_bass_kernel_spmd

NCORES = 8
B, T, C, L = 512, 512, 128, 48
BL = B // NCORES  # 64 examples per core
BLANK = C - 1
H = T // 2  # 256: the fwd/bwd cut
NS = H + 1  # 257 slots per column tile (slot k = value at time k-1)
K = 75.0  # per-step rescale; log K ~= 4.317
F32 = mybir.dt.float32
BF16 = mybir.dt.bfloat16
ALU = mybir.AluOpType
ACTF = mybir.ActivationFunctionType


def build_ctc_program(nc: bass.Bass):
    pgd = nc.dram_tensor("pg", [2 * BL, L * H], BF16, kind="ExternalInput").ap()
    pbd = nc.dram_tensor("pb", [2 * BL, H], BF16, kind="ExternalInput").ap()
    mskd = nc.dram_tensor("msk", [2 * BL, L], F32, kind="ExternalInput").ap()
    out = nc.dram_tensor("out", [BL, 1], F32, kind="ExternalOutput").ap()

    with tile.TileContext(nc) as tc:
        _ctc_body(nc, tc, pgd, pbd, mskd, out)
    return out


def _ctc_body(nc, tc, pgd, pbd, mskd, out):
    P2 = 2 * BL  # 128 partitions: fwd examples | bwd examples

    with (
        tc.tile_pool(name="const", bufs=1) as cpool,
        tc.tile_pool(name="fin", bufs=1) as fpool,
    ):
        # ---- inputs ------------------------------------------------------
        # pbshc[p, k] = blank prob at step k-1 of this half; slot 0 = 1.0
        # (first in queue: scanA_0 only needs this)
        pbshc = cpool.tile([P2, NS], BF16)
        nc.sync.dma_start(out=pbshc[:, 1:NS], in_=pbd[:, :])
        nc.gpsimd.memset(pbshc[:, 0:1], 1.0)

        # pg mega tile: column j at [:, j*H:(j+1)*H]; column 0 rides right
        # behind pb so scanL_0 starts ASAP; the rest (and msk, first needed
        # by the column-1 Act multiply) stream under the wavefront
        pgm = cpool.tile([P2, L * H], BF16)
        nc.sync.dma_start(out=pgm[:, 0:H], in_=pgd[:, 0:H])

        # mc[p, j] = m - 1 in {0,-1}: x = atilde + (m-1)*lprev (skip correction)
        mc = cpool.tile([P2, L], F32)
        nc.sync.dma_start(out=mc[:], in_=mskd[:, :])

        c0 = 1
        for w in (1, 1, 1, 4, 8, 8, 8, 8, 8):
            nc.sync.dma_start(
                out=pgm[:, c0 * H : (c0 + w) * H],
                in_=pgd[:, c0 * H : (c0 + w) * H],
            )
            c0 += w
        assert c0 == L

        # touch Ln once so its table loads during startup slack
        warm = cpool.tile([BL, 1], F32)
        nc.vector.memset(warm[:], 1.0)
        nc.scalar.activation(out=warm[:], in_=warm[:], func=ACTF.Ln)

        # ---- column storage: mega tiles (slot 256 read back at the merge)
        amega = cpool.tile([P2, (L + 1) * NS], BF16)
        lmega = cpool.tile([P2, L * NS], BF16)
        xmega = cpool.tile([P2, L * NS], BF16)
        zcol = cpool.tile([P2, NS], BF16)
        nc.gpsimd.memset(zcol[:], 0.0)
        # bwpack[64:128, i] = backward boundary values, packed in column
        # order: [pb256*btilde_bl 49 | bhat_l 48] (pb256 folded into the
        # blank copies via per-partition Act scale; scale APs must be f32)
        bwpack = cpool.tile([P2, 2 * L + 1], BF16)
        pb256f = cpool.tile([P2, 1], F32)
        nc.vector.tensor_copy(out=pb256f[:], in_=pbshc[:, NS - 1 : NS])

        # ---- packed bidirectional wavefront ------------------------------
        # Per column: DVE does scanA, a 2x-mode TT add, scanL. The skip
        # correction's multiply (mcl = mc*lprev) runs on the idle Activation
        # engine (per-partition scale), overlapped under DVE's scanA, so the
        # DVE chain per column is scan + cheap TT + scan instead of
        # scan + 1x STT + scan. Column 0 has lprev = 0, so its x IS atilde
        # and both the TT and the Act multiply are skipped.
        lprev = zcol
        for j in range(L + 1):
            # top: fwd atilde_j[k] = a_j(k-1) + l_{j-1}(k-1)
            # bottom: bwd btilde for bl_{48-j} (same recurrence, reversed data)
            acol = amega[:, j * NS : (j + 1) * NS]
            nc.vector.tensor_tensor_scan(
                out=acol[:, j:NS], data0=pbshc[:, j:NS], data1=lprev[:, j:NS],
                initial=1.0 if j == 0 else 0.0, op0=ALU.mult, op1=ALU.add,
            )
            # bwd blank boundary: pb256 * btilde_bl_{48-j}[255]
            nc.scalar.activation(
                out=bwpack[BL:P2, L - j : L - j + 1],
                in_=acol[BL:P2, NS - 2 : NS - 1], func=ACTF.Copy,
                scale=pb256f[BL:P2, :],
            )
            if j == L:
                break

            # x = atilde + (m-1)*lprev (skip correction)
            if j == 0:
                x = acol
                # the merge reads x_0[256] from xmega; mirror it there
                nc.scalar.activation(
                    out=xmega[0:BL, NS - 1 : NS],
                    in_=acol[0:BL, NS - 1 : NS], func=ACTF.Copy,
                )
            else:
                x = xmega[:, j * NS : (j + 1) * NS]
                nc.vector.tensor_tensor(
                    out=x[:, j:NS], in0=x[:, j:NS], in1=acol[:, j:NS],
                    op=ALU.add,
                )

            # top: fwd l_j; bottom: bwd l_{47-j}
            lcol = lmega[:, j * NS : (j + 1) * NS]
            nc.vector.tensor_tensor_scan(
                out=lcol[:, j + 1 : NS], data0=x[:, j:H],
                data1=pgm[:, j * H + j : (j + 1) * H],
                initial=0.0, op0=ALU.add, op1=ALU.mult,
            )
            # bwd label boundary: bhat_l_{47-j}(tau=255), raw slot 256
            nc.scalar.activation(
                out=bwpack[BL:P2, L + 1 + L - 1 - j : L + 1 + L - j],
                in_=lcol[BL:P2, NS - 1 : NS], func=ACTF.Copy,
            )
            # mcl for the NEXT column: (m-1)*l_j on the Activation engine,
            # hidden under the next scanA
            if j + 1 <= L - 1:
                xn = xmega[:, (j + 1) * NS : (j + 2) * NS]
                nc.scalar.activation(
                    out=xn[:, j + 1 : NS], in_=lcol[:, j + 1 : NS],
                    func=ACTF.Copy, scale=mc[:, j + 1 : j + 2],
                )
            if j == L - 1:
                # hoist the merge's label-inflow combine into the gap before
                # the final blank scan: ul_j = l_j(255) + x_j[256]
                l3w = lmega[:].rearrange("p (j s) -> p j s", s=NS)
                x3w = xmega[:].rearrange("p (j s) -> p j s", s=NS)
                ul = fpool.tile([BL, L], F32)
                ul3 = ul[:].rearrange("p (j s) -> p j s", s=1)
                nc.vector.tensor_tensor(
                    out=ul3[:], in0=l3w[0:BL, :, NS - 1 : NS],
                    in1=x3w[0:BL, :, NS - 1 : NS], op=ALU.add,
                )
            lprev = lcol

        # ---- merge at the cut --------------------------------------------
        # move backward boundary vector to the forward partitions: DVE
        # stream shuffle (identity mask, base-partition-64 view) instead of
        # an SBUF-to-SBUF DMA round trip (saves ~1.7us of DMA+sem latency)
        NB = 2 * L + 1  # 97: [pb256*btilde_bl 49 | bhat_l 48]
        shuf = fpool.tile([BL, NB], BF16)
        nc.vector.stream_shuffle(
            out=shuf[:], in_=bwpack[BL:P2, :], mask=list(range(32))
        )

        a3 = amega[:].rearrange("p (j s) -> p j s", s=NS)
        s3 = shuf[:].rearrange("p (j s) -> p j s", s=1)
        ul3 = ul[:].rearrange("p (j s) -> p j s", s=1)

        # prod[:, 0:49] = atilde_j[256] * (pb256*btilde_bl_j[255])
        # prod[:, 49:97] = (l_j(255) + x_j[256]) * bhat_l_j(255)
        prod = fpool.tile([BL, NB], F32)
        p3 = prod[:].rearrange("p (j s) -> p j s", s=1)
        nc.vector.tensor_tensor(
            out=p3[:, 0 : L + 1, :], in0=a3[0:BL, :, NS - 1 : NS],
            in1=s3[:, 0 : L + 1, :], op=ALU.mult,
        )
        nc.vector.tensor_tensor(
            out=p3[:, L + 1 : NB, :], in0=ul3[:], in1=s3[:, L + 1 : NB, :],
            op=ALU.mult,
        )
        # P = sum of all 97 terms
        z = fpool.tile([BL, 1], F32)
        nc.vector.tensor_reduce(out=z[:], in_=prod[:], axis=mybir.AxisListType.X, op=ALU.add)

        # ---- finalize: loss = T*log K - log P ----------------------------
        logz = fpool.tile([BL, 1], F32)
        nc.scalar.activation(out=logz[:], in_=z[:], func=ACTF.Ln)
        loss = fpool.tile([BL, 1], F32)
        nc.scalar.activation(
            out=loss[:], in_=logz[:], func=ACTF.Copy,
            scale=-1.0, bias=float(T * math.log(K)),
        )
        nc.sync.dma_start(out=out[:, :], in_=loss[:])


_CACHE: dict = {}


def _get_program():
    if "nc" not in _CACHE:
        nc = bacc.Bacc("TRN2", target_bir_lowering=False, debug=False)
        build_ctc_program(nc)
        nc.compile()
        _CACHE["nc"] = nc
    return _CACHE["nc"]


def kernel(y_true: np.ndarray, y_pred: np.ndarray) -> np.ndarray:
    nc = _get_program()
    lab = np.ascontiguousarray(np.asarray(y_true).astype(np.int32))  # [B, L]
    yp = np.asarray(y_pred, dtype=np.float32)  # [B, T, C]
    # input conditioning: constant K rescale folded into the bf16 quantization,
    # fwd half t=0..255 / reversed bwd half t=511..256 per partition group
    yp2 = (K * yp).astype(ml_dtypes.bfloat16)
    fwd = yp2[:, :H, :]  # [B, 256, C], time-major
    bwd = yp2[:, H:, :][:, ::-1, :]

    labc = lab.reshape(NCORES, BL, L)
    # host-side gather: pg_top[core, b, j, k] = fwd[b, k, lab[b, j]]
    fwdc = fwd.reshape(NCORES, BL, H, C)
    bwdc = bwd.reshape(NCORES, BL, H, C)
    idx_top = labc[:, :, None, :]  # [NC, BL, 1, L]
    idx_bot = labc[:, :, ::-1][:, :, None, :]
    pg_top = np.take_along_axis(fwdc, idx_top, axis=3)  # [NC, BL, H, L]
    pg_bot = np.take_along_axis(bwdc, idx_bot, axis=3)
    pg_top = pg_top.transpose(0, 1, 3, 2)  # [NC, BL, L, H]
    pg_bot = pg_bot.transpose(0, 1, 3, 2)

    pb_top = fwdc[:, :, :, BLANK]  # [NC, BL, H]
    pb_bot = bwdc[:, :, :, BLANK]

    m = np.zeros((B, L), dtype=np.float32)
    m[:, 1:] = (lab[:, 1:] != lab[:, :-1]).astype(np.float32)
    mc_top = m - 1.0  # mneg_j (col 0 unused: lprev_0 = 0)
    mc_bot = np.zeros((B, L), dtype=np.float32)
    mc_bot[:, 1:] = m[:, :0:-1] - 1.0  # col j>=1: m[:, L-j] - 1
    mct = mc_top.reshape(NCORES, BL, L)
    mcb = mc_bot.reshape(NCORES, BL, L)

    in_maps = [
        {
            "pg": np.ascontiguousarray(
                np.concatenate([pg_top[c], pg_bot[c]], axis=0).reshape(2 * BL, L * H)
            ),
            "pb": np.ascontiguousarray(
                np.concatenate([pb_top[c], pb_bot[c]], axis=0)
            ),
            "msk": np.ascontiguousarray(np.concatenate([mct[c], mcb[c]], axis=0)),
        }
        for c in range(NCORES)
    ]
    res = run_bass_kernel_spmd(nc, in_maps, list(range(NCORES)))
    return np.concatenate([res.results[c]["out"] for c in range(NCORES)], axis=0)


# revision 25
# speedup vs baseline: 1.0017x; 1.0009x over previous
"""CTC loss (keras ctc_batch_cost semantics) on Trainium2, 8-core data parallel.

Bidirectional packed wavefront (per core, 64 examples on 128 partitions):
  Linear-domain CTC with per-step rescale K (p' = K*p, loss = T*log K -
  log P). The lattice is split at t = T/2: partitions 0-63 run the FORWARD
  wavefront over t in [0, 256) (columns bl_0, l_0, bl_1, ...), partitions
  64-127 run the BACKWARD wavefront over reversed time tau = 511-t
  (columns bl_48, l_47, bl_47, ...). Both recurrences have the identical
  "atilde" scan form (state = pbsh*state + lprev; x = atilde + (m-1)*lprev;
  l-scan state = (x + state)*pg), so each wavefront step is two [128, ~233]
  DVE tensor_tensor_scans plus the skip correction. Column i is zero for
  k < i in both directions (head pruning).

  Skip correction: the multiply mcl = (m-1)*l_j runs on the otherwise-idle
  Activation engine (Copy with per-partition scale), overlapped under the
  next column's blank scan; DVE then only pays a 2x-mode TensorTensor add
  (x = mcl + atilde) instead of a 1x scalar_tensor_tensor.

  Merge at the cut: P = sum_{s'} I_{s'} * bhat_{s'}(256), where I_{s'} is
  the forward pre-multiply inflow at t=256 (atilde/x/l slots 256) and
  bhat_{s'}(256) comes from the backward tiles (slot 255 * pb256 for
  blanks — pb256 folded into the boundary copies via Act scale — and raw
  slot 256 for labels). Boundary values are packed by tiny Act copies during
  the wavefront, moved across partitions with one DVE StreamShuffle
  (identity mask over a base-partition-64 view; no DMA round trip), and
  combined with two strided TTs and a single reduce.

  Data movement: the host gathers the per-column label probabilities and the
  blank row directly (pg[p, j, k] = bf16(K * y_pred[b, t, lab]) in wavefront
  layout, h=0: t=k forward / h=1: t=511-k backward), so the device streams
  plain contiguous DMAs — no indirect gathers (a Pool-engine SWDGE gather
  costs ~1us per column and was the original bottleneck). pb and pg column 0
  are queued first so the wavefront starts ~3us in; the remaining columns
  stream under it in growing chunks.

Shapes are hardcoded for B=512, T=512, C=128, L=48 (S=97), 8 cores.
"""

import sys

if "/opt/trn_rl_repo" not in sys.path:
    sys.path.insert(0, "/opt/trn_rl_repo")

import math

import ml_dtypes
import numpy as np

import concourse.bacc as bacc
import concourse.bass as bass
import concourse.tile as tile
from concourse import mybir
from concourse.bass_utils import run_bass_kernel_spmd

NCORES = 8
B, T, C, L = 512, 512, 128, 48
BL = B // NCORES  # 64 examples per core
BLANK = C - 1
H = T // 2  # 256: the fwd/bwd cut
NS = H + 1  # 257 slots per column tile (slot k = value at time k-1)
K = 75.0  # per-step rescale; log K ~= 4.317
F32 = mybir.dt.float32
BF16 = mybir.dt.bfloat16
ALU = mybir.AluOpType
ACTF = mybir.ActivationFunctionType


def build_ctc_program(nc: bass.Bass):
    pgd = nc.dram_tensor("pg", [2 * BL, L * H], BF16, kind="ExternalInput").ap()
    pbd = nc.dram_tensor("pb", [2 * BL, H], BF16, kind="ExternalInput").ap()
    mskd = nc.dram_tensor("msk", [2 * BL, L], F32, kind="ExternalInput").ap()
    out = nc.dram_tensor("out", [BL, 1], F32, kind="ExternalOutput").ap()

    with tile.TileContext(nc) as tc:
        _ctc_body(nc, tc, pgd, pbd, mskd, out)
    return out


def _ctc_body(nc, tc, pgd, pbd, mskd, out):
    P2 = 2 * BL  # 128 partitions: fwd examples | bwd examples

    with (
        tc.tile_pool(name="const", bufs=1) as cpool,
        tc.tile_pool(name="fin", bufs=1) as fpool,
    ):
        # ---- inputs ------------------------------------------------------
        # pbshc[p, k] = blank prob at step k-1 of this half; slot 0 = 1.0
        # (first in queue: scanA_0 only needs this)
        pbshc = cpool.tile([P2, NS], BF16)
        nc.sync.dma_start(out=pbshc[:, 1:NS], in_=pbd[:, :])
        nc.gpsimd.memset(pbshc[:, 0:1], 1.0)

        # pg mega tile: column j at [:, j*H:(j+1)*H]; column 0 rides right
        # behind pb so scanL_0 starts ASAP; the rest (and msk, first needed
        # by the column-1 Act multiply) stream under the wavefront
        pgm = cpool.tile([P2, L * H], BF16)
        nc.sync.dma_start(out=pgm[:, 0:H], in_=pgd[:, 0:H])

        # mc[p, j] = m - 1 in {0,-1}: x = atilde + (m-1)*lprev (skip correction)
        mc = cpool.tile([P2, L], F32)
        nc.sync.dma_start(out=mc[:], in_=mskd[:, :])

        c0 = 1
        for w in (1, 1, 1, 4, 8, 8, 8, 8, 8):
            nc.sync.dma_start(
                out=pgm[:, c0 * H : (c0 + w) * H],
                in_=pgd[:, c0 * H : (c0 + w) * H],
            )
            c0 += w
        assert c0 == L

        # touch Ln once so its table loads during startup slack
        warm = cpool.tile([BL, 1], F32)
        nc.vector.memset(warm[:], 1.0)
        nc.scalar.activation(out=warm[:], in_=warm[:], func=ACTF.Ln)

        # ---- column storage: mega tiles (slot 256 read back at the merge)
        # amega has L extra phantom columns: the merge's label-inflow terms
        # (ul_j = l_j(255) + x_j[256]) are written into their slot-(NS-1)
        # positions so the final combine reads ONE 97-entry NS-strided AP
        amega = cpool.tile([P2, (2 * L + 1) * NS], BF16)
        lmega = cpool.tile([P2, L * NS], BF16)
        xmega = cpool.tile([P2, L * NS], BF16)
        zcol = cpool.tile([P2, NS], BF16)
        nc.gpsimd.memset(zcol[:], 0.0)
        # bwpack[64:128, i] = backward boundary values, packed in column
        # order: [pb256*btilde_bl 49 | bhat_l 48] (pb256 folded into the
        # blank copies via per-partition Act scale; scale APs must be f32)
        bwpack = cpool.tile([P2, 2 * L + 1], BF16)
        pb256f = cpool.tile([P2, 1], F32)
        nc.vector.tensor_copy(out=pb256f[:], in_=pbshc[:, NS - 1 : NS])

        # ---- packed bidirectional wavefront ------------------------------
        # Per column: DVE does scanA, a 2x-mode TT add, scanL. The skip
        # correction's multiply (mcl = mc*lprev) runs on the idle Activation
        # engine (per-partition scale), overlapped under DVE's scanA, so the
        # DVE chain per column is scan + cheap TT + scan instead of
        # scan + 1x STT + scan. Column 0 has lprev = 0, so its x IS atilde
        # and both the TT and the Act multiply are skipped.
        lprev = zcol
        for j in range(L + 1):
            # top: fwd atilde_j[k] = a_j(k-1) + l_{j-1}(k-1)
            # bottom: bwd btilde for bl_{48-j} (same recurrence, reversed data)
            acol = amega[:, j * NS : (j + 1) * NS]
            nc.vector.tensor_tensor_scan(
                out=acol[:, j:NS], data0=pbshc[:, j:NS], data1=lprev[:, j:NS],
                initial=1.0 if j == 0 else 0.0, op0=ALU.mult, op1=ALU.add,
            )
            # bwd blank boundary: pb256 * btilde_bl_{48-j}[255]
            nc.scalar.activation(
                out=bwpack[BL:P2, L - j : L - j + 1],
                in_=acol[BL:P2, NS - 2 : NS - 1], func=ACTF.Copy,
                scale=pb256f[BL:P2, :],
            )
            if j == L:
                break

            # x = atilde + (m-1)*lprev (skip correction)
            if j == 0:
                x = acol
                # the merge reads x_0[256] from xmega; mirror it there
                nc.scalar.activation(
                    out=xmega[0:BL, NS - 1 : NS],
                    in_=acol[0:BL, NS - 1 : NS], func=ACTF.Copy,
                )
            else:
                x = xmega[:, j * NS : (j + 1) * NS]
                nc.vector.tensor_tensor(
                    out=x[:, j:NS], in0=x[:, j:NS], in1=acol[:, j:NS],
                    op=ALU.add,
                )

            # top: fwd l_j; bottom: bwd l_{47-j}
            lcol = lmega[:, j * NS : (j + 1) * NS]
            nc.vector.tensor_tensor_scan(
                out=lcol[:, j + 1 : NS], data0=x[:, j:H],
                data1=pgm[:, j * H + j : (j + 1) * H],
                initial=0.0, op0=ALU.add, op1=ALU.mult,
            )
            # bwd label boundary: bhat_l_{47-j}(tau=255), raw slot 256
            nc.scalar.activation(
                out=bwpack[BL:P2, L + 1 + L - 1 - j : L + 1 + L - j],
                in_=lcol[BL:P2, NS - 1 : NS], func=ACTF.Copy,
            )
            # mcl for the NEXT column: (m-1)*l_j on the Activation engine,
            # hidden under the next scanA
            if j + 1 <= L - 1:
                xn = xmega[:, (j + 1) * NS : (j + 2) * NS]
                nc.scalar.activation(
                    out=xn[:, j + 1 : NS], in_=lcol[:, j + 1 : NS],
                    func=ACTF.Copy, scale=mc[:, j + 1 : j + 2],
                )
            if j == L - 1:
                # hoist the merge's label-inflow combine into the gap before
                # the final blank scan: ul_j = l_j(255) + x_j[256], written
                # strided into amega's phantom columns L+1..2L
                l3w = lmega[:].rearrange("p (j s) -> p j s", s=NS)
                x3w = xmega[:].rearrange("p (j s) -> p j s", s=NS)
                a3w = amega[:].rearrange("p (j s) -> p j s", s=NS)
                nc.vector.tensor_tensor(
                    out=a3w[0:BL, L + 1 : 2 * L + 1, NS - 1 : NS],
                    in0=l3w[0:BL, :, NS - 1 : NS],
                    in1=x3w[0:BL, :, NS - 1 : NS], op=ALU.add,
                )
            lprev = lcol

        # ---- merge at the cut --------------------------------------------
        # move backward boundary vector to the forward partitions: DVE
        # stream shuffle (identity mask, base-partition-64 view) instead of
        # an SBUF-to-SBUF DMA round trip (saves ~1.7us of DMA+sem latency)
        NB = 2 * L + 1  # 97: [pb256*btilde_bl 49 | bhat_l 48]
        shuf = fpool.tile([BL, NB], BF16)
        nc.vector.stream_shuffle(
            out=shuf[:], in_=bwpack[BL:P2, :], mask=list(range(32))
        )

        a3 = amega[:].rearrange("p (j s) -> p j s", s=NS)
        s3 = shuf[:].rearrange("p (j s) -> p j s", s=1)

        # prod[:, 0:49] = atilde_j[256] * (pb256*btilde_bl_j[255])
        # prod[:, 49:97] = (l_j(255) + x_j[256]) * bhat_l_j(255)
        # — one TT over the single NS-strided 97-entry amega AP
        prod = fpool.tile([BL, NB], F32)
        p3 = prod[:].rearrange("p (j s) -> p j s", s=1)
        nc.vector.tensor_tensor(
            out=p3[:], in0=a3[0:BL, 0:NB, NS - 1 : NS], in1=s3[:], op=ALU.mult,
        )
        # P = sum of all 97 terms
        z = fpool.tile([BL, 1], F32)
        nc.vector.tensor_reduce(out=z[:], in_=prod[:], axis=mybir.AxisListType.X, op=ALU.add)

        # ---- finalize: loss = T*log K - log P ----------------------------
        logz = fpool.tile([BL, 1], F32)
        nc.scalar.activation(out=logz[:], in_=z[:], func=ACTF.Ln)
        loss = fpool.tile([BL, 1], F32)
        nc.scalar.activation(
            out=loss[:], in_=logz[:], func=ACTF.Copy,
            scale=-1.0, bias=float(T * math.log(K)),
        )
        nc.sync.dma_start(out=out[:, :], in_=loss[:])


_CACHE: dict = {}


def _get_program():
    if "nc" not in _CACHE:
        nc = bacc.Bacc("TRN2", target_bir_lowering=False, debug=False)
        build_ctc_program(nc)
        nc.compile()
        _CACHE["nc"] = nc
    return _CACHE["nc"]


def kernel(y_true: np.ndarray, y_pred: np.ndarray) -> np.ndarray:
    nc = _get_program()
    lab = np.ascontiguousarray(np.asarray(y_true).astype(np.int32))  # [B, L]
    yp = np.asarray(y_pred, dtype=np.float32)  # [B, T, C]
    # input conditioning: constant K rescale folded into the bf16 quantization,
    # fwd half t=0..255 / reversed bwd half t=511..256 per partition group
    yp2 = (K * yp).astype(ml_dtypes.bfloat16)
    fwd = yp2[:, :H, :]  # [B, 256, C], time-major
    bwd = yp2[:, H:, :][:, ::-1, :]

    labc = lab.reshape(NCORES, BL, L)
    # host-side gather: pg_top[core, b, j, k] = fwd[b, k, lab[b, j]]
    fwdc = fwd.reshape(NCORES, BL, H, C)
    bwdc = bwd.reshape(NCORES, BL, H, C)
    idx_top = labc[:, :, None, :]  # [NC, BL, 1, L]
    idx_bot = labc[:, :, ::-1][:, :, None, :]
    pg_top = np.take_along_axis(fwdc, idx_top, axis=3)  # [NC, BL, H, L]
    pg_bot = np.take_along_axis(bwdc, idx_bot, axis=3)
    pg_top = pg_top.transpose(0, 1, 3, 2)  # [NC, BL, L, H]
    pg_bot = pg_bot.transpose(0, 1, 3, 2)

    pb_top = fwdc[:, :, :, BLANK]  # [NC, BL, H]
    pb_bot = bwdc[:, :, :, BLANK]

    m = np.zeros((B, L), dtype=np.float32)
    m[:, 1:] = (lab[:, 1:] != lab[:, :-1]).astype(np.float32)
    mc_top = m - 1.0  # mneg_j (col 0 unused: lprev_0 = 0)
    mc_bot = np.zeros((B, L), dtype=np.float32)
    mc_bot[:, 1:] = m[:, :0:-1] - 1.0  # col j>=1: m[:, L-j] - 1
    mct = mc_top.reshape(NCORES, BL, L)
    mcb = mc_bot.reshape(NCORES, BL, L)

    in_maps = [
        {
            "pg": np.ascontiguousarray(
                np.concatenate([pg_top[c], pg_bot[c]], axis=0).reshape(2 * BL, L * H)
            ),
            "pb": np.ascontiguousarray(
                np.concatenate([pb_top[c], pb_bot[c]], axis=0)
            ),
            "msk": np.ascontiguousarray(np.concatenate([mct[c], mcb[c]], axis=0)),
        }
        for c in range(NCORES)
    ]
    res = run_bass_kernel_spmd(nc, in_maps, list(range(NCORES)))
    return np.concatenate([res.results[c]["out"] for c in range(NCORES)], axis=0)


# revision 26
# speedup vs baseline: 1.1718x; 1.1698x over previous
"""CTC loss (keras ctc_batch_cost semantics) on Trainium2, 8-core data parallel.

Label-dimension (s-cut) bidirectional packed wavefront, 64 examples per core
on 128 partitions:

  Linear-domain CTC with per-step rescale K (p' = K*p, loss = T*log K -
  log P). The lattice is split along the LABEL axis at label 24 (0-based):
  partitions 0-63 run the FORWARD wavefront over labels 0..23 with full-T
  (512-step) scans; partitions 64-127 run the BACKWARD wavefront over
  labels 47..24 on time-reversed data. Every CTC path visits every label
  column exactly once, and the only edges crossing the s-cut land in label
  24, so P = sum_t x_24(t) * bhat_24(t): the forward pre-emission inflow
  into label 24 at t (one extra blank scan + TT after the 24 forward
  columns) dotted against the backward completion values (the backward
  side's last label column, time-reversed).

  Versus the time-cut wavefront (48 columns of ~T/2 scans), this halves the
  serial op count (74 vs 146) at the same total element count, halving the
  fixed per-op cost (~250ns dependency latency + SBUF access each) that
  dominated, and the longer scans fully hide the Activation-engine skip
  multiply.

  Per column: DVE tensor_tensor_scan (blank chain, state = pb*s + lprev),
  a 2x-mode TT add for the skip correction x = mcl + atilde where
  mcl = (m-1)*l_prev is computed on the idle Activation engine
  (per-partition scale) under the blank scan, and a second scan
  (label chain, state = (x+s)*pg). Column slices [j:W) implement head
  pruning; unreachable tail values are finite and multiplied by
  backward-side zeros at the merge.

  Merge: one StreamShuffle moves the backward label-24 row to partitions
  0-63 (identity mask over a base-partition-64 view), one TT pairs fwd
  slot k with bwd slot 512-k via a negative-stride AP (t + tau = 511 with
  the emission at t counted by the backward side), one reduce sums the 488
  crossing terms, Ln + affine give the loss.

  Data movement: the host gathers blank/label rows directly in wavefront
  layout (pg[p, j, k] = bf16(K * y_pred[b, t, lab]), forward t=k on top
  partitions / reversed t=511-k on the bottom), so the device streams plain
  contiguous DMAs - no indirect gathers. pb and pg column 0 are queued
  first so the wavefront starts ~3us in; the rest streams under it.

Shapes are hardcoded for B=512, T=512, C=128, L=48 (S=97), 8 cores.
"""

import sys

if "/opt/trn_rl_repo" not in sys.path:
    sys.path.insert(0, "/opt/trn_rl_repo")

import math

import ml_dtypes
import numpy as np

import concourse.bacc as bacc
import concourse.bass as bass
import concourse.tile as tile
from concourse import mybir
from concourse.bass_utils import run_bass_kernel_spmd

NCORES = 8
B, T, C, L = 512, 512, 128, 48
BL = B // NCORES  # 64 examples per core
BLANK = C - 1
LH = L // 2  # 24: labels per direction; the s-cut merge column is label 24
W = T + 1  # 513 slots per column tile (slot k = value at time k-1)
K = 75.0  # per-step rescale; log K ~= 4.317
F32 = mybir.dt.float32
BF16 = mybir.dt.bfloat16
ALU = mybir.AluOpType
ACTF = mybir.ActivationFunctionType


def build_ctc_program(nc: bass.Bass):
    pgd = nc.dram_tensor("pg", [2 * BL, LH * T], BF16, kind="ExternalInput").ap()
    pbd = nc.dram_tensor("pb", [2 * BL, T], BF16, kind="ExternalInput").ap()
    mskd = nc.dram_tensor("msk", [2 * BL, LH + 1], F32, kind="ExternalInput").ap()
    out = nc.dram_tensor("out", [BL, 1], F32, kind="ExternalOutput").ap()

    with tile.TileContext(nc) as tc:
        _ctc_body(nc, tc, pgd, pbd, mskd, out)
    return out


def _ctc_body(nc, tc, pgd, pbd, mskd, out):
    P2 = 2 * BL  # 128 partitions: fwd examples | bwd examples

    with (
        tc.tile_pool(name="const", bufs=1) as cpool,
        tc.tile_pool(name="fin", bufs=1) as fpool,
    ):
        # ---- inputs ------------------------------------------------------
        # pbshc[p, k] = blank prob at time k-1 of this direction; slot 0 = 1
        # (first in queue: scanA_0 only needs this)
        pbshc = cpool.tile([P2, W], BF16)
        nc.sync.dma_start(out=pbshc[:, 1:W], in_=pbd[:, :])
        nc.gpsimd.memset(pbshc[:, 0:1], 1.0)

        # pg mega tile: column j at [:, j*T:(j+1)*T]; column 0 rides right
        # behind pb so scanL_0 starts ASAP; the rest (and msk, first needed
        # by the column-1 Act multiply) stream under the wavefront
        pgm = cpool.tile([P2, LH * T], BF16)
        nc.sync.dma_start(out=pgm[:, 0:T], in_=pgd[:, 0:T])

        # mc[p, j] = m - 1 in {0,-1}: x = atilde + (m-1)*lprev (skip corr.)
        mc = cpool.tile([P2, LH + 1], F32)
        nc.sync.dma_start(out=mc[:], in_=mskd[:, :])

        c0 = 1
        for w in (1, 1, 1, 4, 8, 8):
            nc.sync.dma_start(
                out=pgm[:, c0 * T : (c0 + w) * T],
                in_=pgd[:, c0 * T : (c0 + w) * T],
            )
            c0 += w
        assert c0 == LH

        # touch Ln once so its table loads during startup slack
        warm = cpool.tile([BL, 1], F32)
        nc.vector.memset(warm[:], 1.0)
        nc.scalar.activation(out=warm[:], in_=warm[:], func=ACTF.Ln)

        # ---- column storage ---------------------------------------------
        amega = cpool.tile([P2, (LH + 1) * W], BF16)
        lmega = cpool.tile([P2, LH * W], BF16)
        xmega = cpool.tile([P2, (LH + 1) * W], BF16)
        zcol = cpool.tile([P2, W], BF16)
        nc.gpsimd.memset(zcol[:], 0.0)
        # the merge reads the full backward label-24 row; zero its
        # head-pruned slots (t too late to finish -> contribution 0)
        nc.gpsimd.memset(lmega[:, (LH - 1) * W : (LH - 1) * W + LH], 0.0)

        # ---- packed bidirectional wavefront ------------------------------
        lprev = zcol
        for j in range(LH):
            # top: fwd atilde_j[k] = pb(k-1)*a(k-2...) + l_{j-1}(k-1)
            # bottom: bwd btilde (same recurrence on time-reversed data)
            acol = amega[:, j * W : (j + 1) * W]
            nc.vector.tensor_tensor_scan(
                out=acol[:, j:W], data0=pbshc[:, j:W], data1=lprev[:, j:W],
                initial=1.0 if j == 0 else 0.0, op0=ALU.mult, op1=ALU.add,
            )

            # x = atilde + (m-1)*lprev (skip correction); col 0: x = atilde
            if j == 0:
                x = acol
            else:
                x = xmega[:, j * W : (j + 1) * W]
                nc.vector.tensor_tensor(
                    out=x[:, j:W], in0=x[:, j:W], in1=acol[:, j:W],
                    op=ALU.add,
                )

            # top: fwd l_j; bottom: bwd l (labels from the far end)
            lcol = lmega[:, j * W : (j + 1) * W]
            nc.vector.tensor_tensor_scan(
                out=lcol[:, j + 1 : W], data0=x[:, j:T],
                data1=pgm[:, j * T + j : (j + 1) * T],
                initial=0.0, op0=ALU.add, op1=ALU.mult,
            )
            # mcl for the NEXT column: (m-1)*l_j on the Activation engine,
            # hidden under the next blank scan (also feeds the final TT_24)
            xn = xmega[:, (j + 1) * W : (j + 2) * W]
            nc.scalar.activation(
                out=xn[:, j + 1 : W], in_=lcol[:, j + 1 : W],
                func=ACTF.Copy, scale=mc[:, j + 1 : j + 2],
            )
            lprev = lcol

        # ---- extra forward column: inflow into the merge label (24) ------
        acol24 = amega[:, LH * W : (LH + 1) * W]
        nc.vector.tensor_tensor_scan(
            out=acol24[:, LH:W], data0=pbshc[:, LH:W], data1=lprev[:, LH:W],
            initial=0.0, op0=ALU.mult, op1=ALU.add,
        )
        x24 = xmega[:, LH * W : (LH + 1) * W]
        nc.vector.tensor_tensor(
            out=x24[:, LH:W], in0=x24[:, LH:W], in1=acol24[:, LH:W],
            op=ALU.add,
        )

        # ---- s-cut merge -------------------------------------------------
        # backward label-24 row down to partitions 0-63
        shufb = fpool.tile([BL, W], BF16)
        nc.vector.stream_shuffle(
            out=shufb[:], in_=lmega[BL:P2, (LH - 1) * W : LH * W],
            mask=list(range(32)),
        )
        # P = sum_{t=24..511} x24[t] * bhat[512-t]  (t + tau = 511; the
        # emission at t is counted by the backward side)
        prod = fpool.tile([BL, T - LH], BF16)
        nc.vector.tensor_tensor(
            out=prod[:], in0=x24[0:BL, LH:T], in1=shufb[:, T - LH : 0 : -1],
            op=ALU.mult,
        )
        z = fpool.tile([BL, 1], F32)
        nc.vector.tensor_reduce(
            out=z[:], in_=prod[:], axis=mybir.AxisListType.X, op=ALU.add
        )

        # ---- finalize: loss = T*log K - log P ----------------------------
        logz = fpool.tile([BL, 1], F32)
        nc.scalar.activation(out=logz[:], in_=z[:], func=ACTF.Ln)
        loss = fpool.tile([BL, 1], F32)
        nc.scalar.activation(
            out=loss[:], in_=logz[:], func=ACTF.Copy,
            scale=-1.0, bias=float(T * math.log(K)),
        )
        nc.sync.dma_start(out=out[:, :], in_=loss[:])


_CACHE: dict = {}


def _get_program():
    if "nc" not in _CACHE:
        nc = bacc.Bacc("TRN2", target_bir_lowering=False, debug=False)
        build_ctc_program(nc)
        nc.compile()
        _CACHE["nc"] = nc
    return _CACHE["nc"]


def kernel(y_true: np.ndarray, y_pred: np.ndarray) -> np.ndarray:
    nc = _get_program()
    lab = np.ascontiguousarray(np.asarray(y_true).astype(np.int32))  # [B, L]
    yp = np.asarray(y_pred, dtype=np.float32)  # [B, T, C]
    # input conditioning: constant K rescale folded into the bf16 quantization
    yp2 = (K * yp).astype(ml_dtypes.bfloat16)  # [B, T, C]

    pb_top = yp2[:, :, BLANK]  # [B, T]
    pb_bot = yp2[:, ::-1, BLANK]

    labc = lab.reshape(NCORES, BL, L)
    ypc = yp2.reshape(NCORES, BL, T, C)
    ypc_rev = ypc[:, :, ::-1, :]
    # fwd labels 0..23 at forward time; bwd labels 47..24 at reversed time
    idx_top = labc[:, :, None, 0:LH]  # [NC, BL, 1, 24]
    idx_bot = labc[:, :, ::-1][:, :, None, 0:LH]
    pg_top = np.take_along_axis(ypc, idx_top, axis=3)  # [NC, BL, T, 24]
    pg_bot = np.take_along_axis(ypc_rev, idx_bot, axis=3)
    pg_top = pg_top.transpose(0, 1, 3, 2)  # [NC, BL, 24, T]
    pg_bot = pg_bot.transpose(0, 1, 3, 2)

    m = np.zeros((B, L), dtype=np.float32)
    m[:, 1:] = (lab[:, 1:] != lab[:, :-1]).astype(np.float32)
    mc_top = (m - 1.0)[:, 0 : LH + 1]
    mc_bot = np.zeros((B, L), dtype=np.float32)
    mc_bot[:, 1:] = m[:, :0:-1] - 1.0  # col j>=1: m[:, L-j] - 1
    mc_bot = mc_bot[:, 0 : LH + 1]
    mct = mc_top.reshape(NCORES, BL, LH + 1)
    mcb = mc_bot.reshape(NCORES, BL, LH + 1)
    pbt = pb_top.reshape(NCORES, BL, T)
    pbb = pb_bot.reshape(NCORES, BL, T)

    in_maps = [
        {
            "pg": np.ascontiguousarray(
                np.concatenate([pg_top[c], pg_bot[c]], axis=0).reshape(
                    2 * BL, LH * T
                )
            ),
            "pb": np.ascontiguousarray(np.concatenate([pbt[c], pbb[c]], axis=0)),
            "msk": np.ascontiguousarray(np.concatenate([mct[c], mcb[c]], axis=0)),
        }
        for c in range(NCORES)
    ]
    res = run_bass_kernel_spmd(nc, in_maps, list(range(NCORES)))
    return np.concatenate([res.results[c]["out"] for c in range(NCORES)], axis=0)


# revision 27
# speedup vs baseline: 1.1807x; 1.0076x over previous
"""CTC loss (keras ctc_batch_cost semantics) on Trainium2, 8-core data parallel.

Label-dimension (s-cut) bidirectional packed wavefront, 64 examples per core
on 128 partitions:

  Linear-domain CTC with per-step rescale K (p' = K*p, loss = T*log K -
  log P). The lattice is split along the LABEL axis at label 24 (0-based):
  partitions 0-63 run the FORWARD wavefront over labels 0..23 with full-T
  (512-step) scans; partitions 64-127 run the BACKWARD wavefront over
  labels 47..24 on time-reversed data. Every CTC path visits every label
  column exactly once, and the only edges crossing the s-cut land in label
  24, so P = sum_t x_24(t) * bhat_24(t): the forward pre-emission inflow
  into label 24 at t (one extra blank scan + TT after the 24 forward
  columns) dotted against the backward completion values (the backward
  side's last label column, time-reversed).

  Versus the time-cut wavefront (48 columns of ~T/2 scans), this halves the
  serial op count (74 vs 146) at the same total element count, halving the
  fixed per-op cost (~250ns dependency latency + SBUF access each) that
  dominated, and the longer scans fully hide the Activation-engine skip
  multiply.

  Per column: DVE tensor_tensor_scan (blank chain, state = pb*s + lprev),
  a 2x-mode TT add for the skip correction x = mcl + atilde where
  mcl = (m-1)*l_prev is computed on the idle Activation engine
  (per-partition scale) under the blank scan, and a second scan
  (label chain, state = (x+s)*pg). Column slices [j:W) implement head
  pruning; unreachable tail values are finite and multiplied by
  backward-side zeros at the merge.

  Merge: one StreamShuffle moves the backward label-24 row to partitions
  0-63 (identity mask over a base-partition-64 view), one TT pairs fwd
  slot k with bwd slot 512-k via a negative-stride AP (t + tau = 511 with
  the emission at t counted by the backward side), one reduce sums the 488
  crossing terms, Ln + affine give the loss.

  Data movement: the host gathers blank/label rows directly in wavefront
  layout (pg[p, j, k] = bf16(K * y_pred[b, t, lab]), forward t=k on top
  partitions / reversed t=511-k on the bottom), so the device streams plain
  contiguous DMAs - no indirect gathers. pb and pg column 0 are queued
  first so the wavefront starts ~3us in; the rest streams under it.

Shapes are hardcoded for B=512, T=512, C=128, L=48 (S=97), 8 cores.
"""

import sys

if "/opt/trn_rl_repo" not in sys.path:
    sys.path.insert(0, "/opt/trn_rl_repo")

import math

import ml_dtypes
import numpy as np

import concourse.bacc as bacc
import concourse.bass as bass
import concourse.tile as tile
from concourse import mybir
from concourse.bass_utils import run_bass_kernel_spmd

NCORES = 8
B, T, C, L = 512, 512, 128, 48
BL = B // NCORES  # 64 examples per core
BLANK = C - 1
LH = L // 2  # 24: labels per direction; the s-cut merge column is label 24
W = T + 1  # 513 slots per column tile (slot k = value at time k-1)
K = 75.0  # per-step rescale; log K ~= 4.317
F32 = mybir.dt.float32
BF16 = mybir.dt.bfloat16
ALU = mybir.AluOpType
ACTF = mybir.ActivationFunctionType


def build_ctc_program(nc: bass.Bass):
    pgd = nc.dram_tensor("pg", [2 * BL, LH * T], BF16, kind="ExternalInput").ap()
    pbd = nc.dram_tensor("pb", [2 * BL, T], BF16, kind="ExternalInput").ap()
    mskd = nc.dram_tensor("msk", [2 * BL, LH + 1], F32, kind="ExternalInput").ap()
    out = nc.dram_tensor("out", [BL, 1], F32, kind="ExternalOutput").ap()

    with tile.TileContext(nc) as tc:
        _ctc_body(nc, tc, pgd, pbd, mskd, out)
    return out


def _ctc_body(nc, tc, pgd, pbd, mskd, out):
    P2 = 2 * BL  # 128 partitions: fwd examples | bwd examples

    with (
        tc.tile_pool(name="const", bufs=1) as cpool,
        tc.tile_pool(name="fin", bufs=1) as fpool,
    ):
        # ---- inputs ------------------------------------------------------
        # pbshc[p, k] = blank prob at time k-1 of this direction; slot 0 = 1
        # (first in queue: scanA_0 only needs this)
        pbshc = cpool.tile([P2, W], BF16)
        nc.sync.dma_start(out=pbshc[:, 1:W], in_=pbd[:, :])
        nc.gpsimd.memset(pbshc[:, 0:1], 1.0)

        # pg mega tile: column j at [:, j*T:(j+1)*T]; column 0 rides right
        # behind pb so scanL_0 starts ASAP; the rest (and msk, first needed
        # by the column-1 Act multiply) stream under the wavefront
        pgm = cpool.tile([P2, LH * T], BF16)
        nc.sync.dma_start(out=pgm[:, 0:T], in_=pgd[:, 0:T])

        # mc[p, j] = m - 1 in {0,-1}: x = atilde + (m-1)*lprev (skip corr.)
        mc = cpool.tile([P2, LH + 1], F32)
        nc.sync.dma_start(out=mc[:], in_=mskd[:, :])

        c0 = 1
        for w in (1, 1, 1, 4, 8, 8):
            nc.sync.dma_start(
                out=pgm[:, c0 * T : (c0 + w) * T],
                in_=pgd[:, c0 * T : (c0 + w) * T],
            )
            c0 += w
        assert c0 == LH

        # touch Ln once so its table loads during startup slack
        warm = cpool.tile([BL, 1], F32)
        nc.vector.memset(warm[:], 1.0)
        nc.scalar.activation(out=warm[:], in_=warm[:], func=ACTF.Ln)

        # ---- column storage ---------------------------------------------
        amega = cpool.tile([P2, (LH + 1) * W], BF16)
        lmega = cpool.tile([P2, LH * W], BF16)
        xmega = cpool.tile([P2, (LH + 1) * W], BF16)
        zcol = cpool.tile([P2, W], BF16)
        nc.gpsimd.memset(zcol[:], 0.0)
        # the merge reads the full backward label-24 row; zero its
        # head-pruned slots (t too late to finish -> contribution 0)
        nc.gpsimd.memset(lmega[:, (LH - 1) * W : (LH - 1) * W + LH], 0.0)

        # ---- packed bidirectional wavefront ------------------------------
        lprev = zcol
        for j in range(LH):
            # top: fwd atilde_j[k] = pb(k-1)*a(k-2...) + l_{j-1}(k-1)
            # bottom: bwd btilde (same recurrence on time-reversed data)
            acol = amega[:, j * W : (j + 1) * W]
            nc.vector.tensor_tensor_scan(
                out=acol[:, j:W], data0=pbshc[:, j:W], data1=lprev[:, j:W],
                initial=1.0 if j == 0 else 0.0, op0=ALU.mult, op1=ALU.add,
            )

            # x = atilde + (m-1)*lprev (skip correction); col 0: x = atilde
            if j == 0:
                x = acol
            else:
                x = xmega[:, j * W : (j + 1) * W]
                nc.vector.tensor_tensor(
                    out=x[:, j:W], in0=x[:, j:W], in1=acol[:, j:W],
                    op=ALU.add,
                )

            # top: fwd l_j; bottom: bwd l (labels from the far end)
            lcol = lmega[:, j * W : (j + 1) * W]
            nc.vector.tensor_tensor_scan(
                out=lcol[:, j + 1 : W], data0=x[:, j:T],
                data1=pgm[:, j * T + j : (j + 1) * T],
                initial=0.0, op0=ALU.add, op1=ALU.mult,
            )
            # mcl for the NEXT column: (m-1)*l_j on the Activation engine,
            # hidden under the next blank scan (also feeds the final TT_24)
            xn = xmega[:, (j + 1) * W : (j + 2) * W]
            nc.scalar.activation(
                out=xn[:, j + 1 : W], in_=lcol[:, j + 1 : W],
                func=ACTF.Copy, scale=mc[:, j + 1 : j + 2],
            )
            lprev = lcol

        # ---- extra forward column: inflow into the merge label (24) ------
        acol24 = amega[:, LH * W : (LH + 1) * W]
        nc.vector.tensor_tensor_scan(
            out=acol24[:, LH:W], data0=pbshc[:, LH:W], data1=lprev[:, LH:W],
            initial=0.0, op0=ALU.mult, op1=ALU.add,
        )
        x24 = xmega[:, LH * W : (LH + 1) * W]
        nc.vector.tensor_tensor(
            out=x24[:, LH:W], in0=x24[:, LH:W], in1=acol24[:, LH:W],
            op=ALU.add,
        )

        # ---- s-cut merge -------------------------------------------------
        # backward label-24 row down to partitions 0-63
        shufb = fpool.tile([BL, W], BF16)
        nc.vector.stream_shuffle(
            out=shufb[:], in_=lmega[BL:P2, (LH - 1) * W : LH * W],
            mask=list(range(32)),
        )
        # P = sum_{t=24..511} x24[t] * bhat[512-t]  (t + tau = 511; the
        # emission at t is counted by the backward side) — one STT with
        # accum_out fuses the product and the reduction
        prod = fpool.tile([BL, T - LH], BF16)
        z = fpool.tile([BL, 1], F32)
        nc.vector.scalar_tensor_tensor(
            out=prod[:], in0=x24[0:BL, LH:T], scalar=1.0,
            in1=shufb[:, T - LH : 0 : -1], op0=ALU.mult, op1=ALU.mult,
            accum_out=z[:],
        )

        # ---- finalize: loss = T*log K - log P ----------------------------
        logz = fpool.tile([BL, 1], F32)
        nc.scalar.activation(out=logz[:], in_=z[:], func=ACTF.Ln)
        loss = fpool.tile([BL, 1], F32)
        nc.scalar.activation(
            out=loss[:], in_=logz[:], func=ACTF.Copy,
            scale=-1.0, bias=float(T * math.log(K)),
        )
        nc.sync.dma_start(out=out[:, :], in_=loss[:])


_CACHE: dict = {}


def _get_program():
    if "nc" not in _CACHE:
        nc = bacc.Bacc("TRN2", target_bir_lowering=False, debug=False)
        build_ctc_program(nc)
        nc.compile()
        _CACHE["nc"] = nc
    return _CACHE["nc"]


def kernel(y_true: np.ndarray, y_pred: np.ndarray) -> np.ndarray:
    nc = _get_program()
    lab = np.ascontiguousarray(np.asarray(y_true).astype(np.int32))  # [B, L]
    yp = np.asarray(y_pred, dtype=np.float32)  # [B, T, C]
    # input conditioning: constant K rescale folded into the bf16 quantization
    yp2 = (K * yp).astype(ml_dtypes.bfloat16)  # [B, T, C]

    pb_top = yp2[:, :, BLANK]  # [B, T]
    pb_bot = yp2[:, ::-1, BLANK]

    labc = lab.reshape(NCORES, BL, L)
    ypc = yp2.reshape(NCORES, BL, T, C)
    ypc_rev = ypc[:, :, ::-1, :]
    # fwd labels 0..23 at forward time; bwd labels 47..24 at reversed time
    idx_top = labc[:, :, None, 0:LH]  # [NC, BL, 1, 24]
    idx_bot = labc[:, :, ::-1][:, :, None, 0:LH]
    pg_top = np.take_along_axis(ypc, idx_top, axis=3)  # [NC, BL, T, 24]
    pg_bot = np.take_along_axis(ypc_rev, idx_bot, axis=3)
    pg_top = pg_top.transpose(0, 1, 3, 2)  # [NC, BL, 24, T]
    pg_bot = pg_bot.transpose(0, 1, 3, 2)

    m = np.zeros((B, L), dtype=np.float32)
    m[:, 1:] = (lab[:, 1:] != lab[:, :-1]).astype(np.float32)
    mc_top = (m - 1.0)[:, 0 : LH + 1]
    mc_bot = np.zeros((B, L), dtype=np.float32)
    mc_bot[:, 1:] = m[:, :0:-1] - 1.0  # col j>=1: m[:, L-j] - 1
    mc_bot = mc_bot[:, 0 : LH + 1]
    mct = mc_top.reshape(NCORES, BL, LH + 1)
    mcb = mc_bot.reshape(NCORES, BL, LH + 1)
    pbt = pb_top.reshape(NCORES, BL, T)
    pbb = pb_bot.reshape(NCORES, BL, T)

    in_maps = [
        {
            "pg": np.ascontiguousarray(
                np.concatenate([pg_top[c], pg_bot[c]], axis=0).reshape(
                    2 * BL, LH * T
                )
            ),
            "pb": np.ascontiguousarray(np.concatenate([pbt[c], pbb[c]], axis=0)),
            "msk": np.ascontiguousarray(np.concatenate([mct[c], mcb[c]], axis=0)),
        }
        for c in range(NCORES)
    ]
    res = run_bass_kernel_spmd(nc, in_maps, list(range(NCORES)))
    return np.concatenate([res.results[c]["out"] for c in range(NCORES)], axis=0)


# revision 31
# speedup vs baseline: 1.2293x; 1.0412x over previous
"""CTC loss (keras ctc_batch_cost semantics) on Trainium2, 8-core data parallel.

Label-dimension (s-cut) bidirectional packed wavefront, 64 examples per core
on 128 partitions:

  Linear-domain CTC with per-step rescale K (p' = K*p, loss = T*log K -
  log P). The lattice is split along the LABEL axis at label 24 (0-based):
  partitions 0-63 run the FORWARD wavefront over labels 0..23 with full-T
  (512-step) scans; partitions 64-127 run the BACKWARD wavefront over
  labels 47..24 on time-reversed data. Every CTC path visits every label
  column exactly once, and the only edges crossing the s-cut land in label
  24, so P = sum_t x_24(t) * bhat_24(t): the forward pre-emission inflow
  into label 24 at t (one extra blank scan + TT after the 24 forward
  columns) dotted against the backward completion values (the backward
  side's last label column, time-reversed).

  Versus the time-cut wavefront (48 columns of ~T/2 scans), this halves the
  serial op count (74 vs 146) at the same total element count, halving the
  fixed per-op cost (~250ns dependency latency + SBUF access each) that
  dominated, and the longer scans fully hide the Activation-engine skip
  multiply.

  Per column: DVE tensor_tensor_scan (blank chain, state = pb*s + lprev),
  a 2x-mode TT add for the skip correction x = mcl + atilde where
  mcl = (m-1)*l_prev is computed on the idle Activation engine
  (per-partition scale) under the blank scan, and a second scan
  (label chain, state = (x+s)*pg). Column slices [j:W) implement head
  pruning; unreachable tail values are finite and multiplied by
  backward-side zeros at the merge.

  Merge: one StreamShuffle moves the backward label-24 row to partitions
  0-63 (identity mask over a base-partition-64 view), one TT pairs fwd
  slot k with bwd slot 512-k via a negative-stride AP (t + tau = 511 with
  the emission at t counted by the backward side), one reduce sums the 488
  crossing terms, Ln + affine give the loss.

  Data movement: the host gathers blank/label rows directly in wavefront
  layout (pg[p, j, k] = bf16(K * y_pred[b, t, lab]), forward t=k on top
  partitions / reversed t=511-k on the bottom), so the device streams plain
  contiguous DMAs - no indirect gathers. pb and pg column 0 are queued
  first so the wavefront starts ~3us in; the rest streams under it.

Shapes are hardcoded for B=512, T=512, C=128, L=48 (S=97), 8 cores.
"""

import sys

if "/opt/trn_rl_repo" not in sys.path:
    sys.path.insert(0, "/opt/trn_rl_repo")

import math

import ml_dtypes
import numpy as np

import concourse.bacc as bacc
import concourse.bass as bass
import concourse.tile as tile
from concourse import mybir
from concourse.bass_utils import run_bass_kernel_spmd

NCORES = 8
B, T, C, L = 512, 512, 128, 48
BL = B // NCORES  # 64 examples per core
BLANK = C - 1
LH = L // 2  # 24: labels per direction; the s-cut merge column is label 24
W = T + 1  # 513 slots per column tile (slot k = value at time k-1)
# tail pruning: column j is unreachable past t = 464+j (the remaining labels
# need the rest of the time), so every column's ops cover the constant-width
# sliding window [j, j+CW). Each scanA reads exactly one slot of the previous
# label column beyond its written range; that slot is genuinely unreachable
# (zero), backed by a strided memset of those 24 single slots.
CW = 466
K = 75.0  # per-step rescale; log K ~= 4.317
F32 = mybir.dt.float32
BF16 = mybir.dt.bfloat16
ALU = mybir.AluOpType
ACTF = mybir.ActivationFunctionType


def build_ctc_program(nc: bass.Bass):
    pgd = nc.dram_tensor("pg", [2 * BL, LH * T], BF16, kind="ExternalInput").ap()
    pbd = nc.dram_tensor("pb", [2 * BL, T], BF16, kind="ExternalInput").ap()
    mskd = nc.dram_tensor("msk", [2 * BL, LH + 1], F32, kind="ExternalInput").ap()
    out = nc.dram_tensor("out", [BL, 1], F32, kind="ExternalOutput").ap()

    with tile.TileContext(nc) as tc:
        _ctc_body(nc, tc, pgd, pbd, mskd, out)
    return out


def _ctc_body(nc, tc, pgd, pbd, mskd, out):
    P2 = 2 * BL  # 128 partitions: fwd examples | bwd examples

    with (
        tc.tile_pool(name="const", bufs=1) as cpool,
        tc.tile_pool(name="fin", bufs=1) as fpool,
    ):
        # ---- inputs ------------------------------------------------------
        # pbshc[p, k] = blank prob at time k-1 of this direction; slot 0 = 1
        # (first in queue: scanA_0 only needs this)
        pbshc = cpool.tile([P2, W], BF16)
        nc.sync.dma_start(out=pbshc[:, 1:W], in_=pbd[:, :])
        nc.gpsimd.memset(pbshc[:, 0:1], 1.0)

        # pg mega tile: column j at [:, j*T:(j+1)*T]; column 0 rides right
        # behind pb so scanL_0 starts ASAP; the rest (and msk, first needed
        # by the column-1 Act multiply) stream under the wavefront
        pgm = cpool.tile([P2, LH * T], BF16)
        nc.sync.dma_start(out=pgm[:, 0:T], in_=pgd[:, 0:T])

        # mc[p, j] = m - 1 in {0,-1}: x = atilde + (m-1)*lprev (skip corr.)
        mc = cpool.tile([P2, LH + 1], F32)
        nc.sync.dma_start(out=mc[:], in_=mskd[:, :])

        c0 = 1
        for w in (1, 1, 1, 4, 8, 8):
            nc.sync.dma_start(
                out=pgm[:, c0 * T : (c0 + w) * T],
                in_=pgd[:, c0 * T : (c0 + w) * T],
            )
            c0 += w
        assert c0 == LH

        # touch Ln once so its table loads during startup slack
        warm = cpool.tile([BL, 1], F32)
        nc.vector.memset(warm[:], 1.0)
        nc.scalar.activation(out=warm[:], in_=warm[:], func=ACTF.Ln)

        # ---- column storage ---------------------------------------------
        amega = cpool.tile([P2, (LH + 1) * W], BF16)
        lmega = cpool.tile([P2, LH * W], BF16)
        xmega = cpool.tile([P2, (LH + 1) * W], BF16)
        zcol = cpool.tile([P2, W], BF16)
        nc.gpsimd.memset(zcol[:], 0.0)
        # zero the one-past-the-window slot of each label column (slot
        # j+CW of lcol_j, read by scanA_{j+1} / mcl_{j+1}): one strided
        # 24-element memset on DVE before the wavefront starts
        nc.vector.memset(
            lmega[:, CW : (LH - 1) * (W + 1) + CW + 1 : W + 1], 0.0
        )

        # ---- packed bidirectional wavefront ------------------------------
        lprev = zcol
        for j in range(LH):
            ej = j + CW  # one-past-the-end slot of column j's window
            # top: fwd atilde_j[k] = pb(k-1)*a(k-2...) + l_{j-1}(k-1)
            # bottom: bwd btilde (same recurrence on time-reversed data)
            acol = amega[:, j * W : (j + 1) * W]
            nc.vector.tensor_tensor_scan(
                out=acol[:, j:ej], data0=pbshc[:, j:ej], data1=lprev[:, j:ej],
                initial=1.0 if j == 0 else 0.0, op0=ALU.mult, op1=ALU.add,
            )

            # x = atilde + (m-1)*lprev (skip correction); col 0: x = atilde
            if j == 0:
                x = acol
            else:
                x = xmega[:, j * W : (j + 1) * W]
                nc.vector.tensor_tensor(
                    out=x[:, j:ej], in0=x[:, j:ej], in1=acol[:, j:ej],
                    op=ALU.add,
                )

            # top: fwd l_j; bottom: bwd l (labels from the far end)
            lcol = lmega[:, j * W : (j + 1) * W]
            nc.vector.tensor_tensor_scan(
                out=lcol[:, j + 1 : ej], data0=x[:, j : ej - 1],
                data1=pgm[:, j * T + j : j * T + ej - 1],
                initial=0.0, op0=ALU.add, op1=ALU.mult,
            )
            # mcl for the NEXT column: (m-1)*l_j on the Activation engine,
            # hidden under the next blank scan (also feeds the final TT_24)
            xn = xmega[:, (j + 1) * W : (j + 2) * W]
            nc.scalar.activation(
                out=xn[:, j + 1 : ej + 1], in_=lcol[:, j + 1 : ej + 1],
                func=ACTF.Copy, scale=mc[:, j + 1 : j + 2],
            )
            lprev = lcol

        # ---- extra forward column: inflow into the merge label (24) ------
        e24 = LH + CW  # 490
        acol24 = amega[:, LH * W : (LH + 1) * W]
        nc.vector.tensor_tensor_scan(
            out=acol24[:, LH:e24], data0=pbshc[:, LH:e24],
            data1=lprev[:, LH:e24],
            initial=0.0, op0=ALU.mult, op1=ALU.add,
        )
        x24 = xmega[:, LH * W : (LH + 1) * W]
        nc.vector.tensor_tensor(
            out=x24[:, LH:e24], in0=x24[:, LH:e24], in1=acol24[:, LH:e24],
            op=ALU.add,
        )

        # ---- s-cut merge -------------------------------------------------
        # backward label-24 row (its live slots [24, 489)) down to
        # partitions 0-63
        shufb = fpool.tile([BL, W], BF16)
        nc.vector.stream_shuffle(
            out=shufb[:, LH : T - LH + 1],
            in_=lmega[BL:P2, (LH - 1) * W + LH : (LH - 1) * W + T - LH + 1],
            mask=list(range(32)),
        )
        # P = sum_{t=24..488} x24[t] * bhat[512-t]  (t + tau = 511; the
        # emission at t is counted by the backward side; t > 488 can't
        # finish) — one STT with accum_out fuses product and reduction
        ND = T - 2 * LH + 1  # 465 crossing terms
        prod = fpool.tile([BL, ND], BF16)
        z = fpool.tile([BL, 1], F32)
        nc.vector.scalar_tensor_tensor(
            out=prod[:], in0=x24[0:BL, LH : LH + ND], scalar=1.0,
            in1=shufb[:, T - LH : LH - 1 : -1], op0=ALU.mult, op1=ALU.mult,
            accum_out=z[:],
        )

        # ---- finalize: loss = T*log K - log P ----------------------------
        logz = fpool.tile([BL, 1], F32)
        nc.scalar.activation(out=logz[:], in_=z[:], func=ACTF.Ln)
        loss = fpool.tile([BL, 1], F32)
        nc.scalar.activation(
            out=loss[:], in_=logz[:], func=ACTF.Copy,
            scale=-1.0, bias=float(T * math.log(K)),
        )
        nc.sync.dma_start(out=out[:, :], in_=loss[:])


_CACHE: dict = {}


def _get_program():
    if "nc" not in _CACHE:
        nc = bacc.Bacc("TRN2", target_bir_lowering=False, debug=False)
        build_ctc_program(nc)
        nc.compile()
        _CACHE["nc"] = nc
    return _CACHE["nc"]


def kernel(y_true: np.ndarray, y_pred: np.ndarray) -> np.ndarray:
    nc = _get_program()
    lab = np.ascontiguousarray(np.asarray(y_true).astype(np.int32))  # [B, L]
    yp = np.asarray(y_pred, dtype=np.float32)  # [B, T, C]
    # input conditioning: constant K rescale folded into the bf16 quantization
    yp2 = (K * yp).astype(ml_dtypes.bfloat16)  # [B, T, C]

    pb_top = yp2[:, :, BLANK]  # [B, T]
    pb_bot = yp2[:, ::-1, BLANK]

    labc = lab.reshape(NCORES, BL, L)
    ypc = yp2.reshape(NCORES, BL, T, C)
    ypc_rev = ypc[:, :, ::-1, :]
    # fwd labels 0..23 at forward time; bwd labels 47..24 at reversed time
    idx_top = labc[:, :, None, 0:LH]  # [NC, BL, 1, 24]
    idx_bot = labc[:, :, ::-1][:, :, None, 0:LH]
    pg_top = np.take_along_axis(ypc, idx_top, axis=3)  # [NC, BL, T, 24]
    pg_bot = np.take_along_axis(ypc_rev, idx_bot, axis=3)
    pg_top = pg_top.transpose(0, 1, 3, 2)  # [NC, BL, 24, T]
    pg_bot = pg_bot.transpose(0, 1, 3, 2)

    m = np.zeros((B, L), dtype=np.float32)
    m[:, 1:] = (lab[:, 1:] != lab[:, :-1]).astype(np.float32)
    mc_top = (m - 1.0)[:, 0 : LH + 1]
    mc_bot = np.zeros((B, L), dtype=np.float32)
    mc_bot[:, 1:] = m[:, :0:-1] - 1.0  # col j>=1: m[:, L-j] - 1
    mc_bot = mc_bot[:, 0 : LH + 1]
    mct = mc_top.reshape(NCORES, BL, LH + 1)
    mcb = mc_bot.reshape(NCORES, BL, LH + 1)
    pbt = pb_top.reshape(NCORES, BL, T)
    pbb = pb_bot.reshape(NCORES, BL, T)

    in_maps = [
        {
            "pg": np.ascontiguousarray(
                np.concatenate([pg_top[c], pg_bot[c]], axis=0).reshape(
                    2 * BL, LH * T
                )
            ),
            "pb": np.ascontiguousarray(np.concatenate([pbt[c], pbb[c]], axis=0)),
            "msk": np.ascontiguousarray(np.concatenate([mct[c], mcb[c]], axis=0)),
        }
        for c in range(NCORES)
    ]
    res = run_bass_kernel_spmd(nc, in_maps, list(range(NCORES)))
    return np.concatenate([res.results[c]["out"] for c in range(NCORES)], axis=0)
